# revision 8
# baseline (speedup 1.0000x reference)
"""Trainium2 Bass kernel for an R-GCN-style GCN layer (basis decomposition).

Reference computation (per relation r, with W_r = sum_b coeff[r,b] * basis[b]):
    out = sum_r segment_sum(inp[src_r] * val_r, dst_r) @ W_r + sum_r bias[r]

Algebraic restructure (4 basis accumulators instead of 16 relation matmuls):
    out[d] = sum_b G_b[d] @ basis[b] + bias_sum
    G_b[d] = sum_{edges e: dst_e = d} (coeff[r_e, b] * val_e) * inp[src_e]

Distribution: output nodes are sharded 8 ways (12500 rows/core); every core
holds the full gather table in its own HBM -> no cross-core communication.

Key performance structure:
  - The per-edge feature gather runs as dma_gather on 4 SWDGE queues
    (queue q is served by GPSIMD Q7 core pair (2q, 2q+1)), one queue per
    src segment, so descriptor generation runs 4-way parallel.
  - Whole datapath in bf16: gather table, masks, matmuls (PSUM stays fp32).
  - 18 X columns per (SB, seg): 16 base bucket columns (one per
    (block-in-SB, 32-dst-group)) plus 2 shared overflow columns (blocks
    0-1 -> col 16, blocks 2-3 -> col 17, edges packed densely); overflow
    matmul masks span the whole 512-wide PSUM bank of their block.
  - Per-(SB, seg) valid-index counts are loaded into a Pool register at
    runtime (per-core data) and passed as num_idxs_reg, so dma_gather
    skips all trailing padding slots (the valid slots form a prefix,
    trailing slots hold idx = -1).
  - Masks are built with 5 DVE ops per SB over 4x32 "virtual columns"
    (16 base + 16 overflow quarters per segment):
        D[p, sc, n]    = iota[n] - ldst[p, sc]          (one tensor_sub)
        M[p, sc, b, n] = (D == 0) * w4[p, sc, b]        (one STT per basis)
    with ldst/w4 packed per virtual column on the host.

Per-core shapes are identical across cores (SPMD); only data differs.
Output is produced transposed per block ([fout, node]) and reassembled on host.
"""
import os
import sys

for _p in ("/opt/trn_rl_repo", "/root/.axon_site/_ro/trn_rl_repo"):
    if os.path.isdir(_p) and _p not in sys.path:
        sys.path.insert(0, _p)

import ml_dtypes
import numpy as np

import concourse.bass as bass
import concourse.tile as tile
from concourse import bacc, mybir
from concourse.bass_utils import run_bass_kernel_spmd

# ---------------- problem constants (hardcoded from spec) ----------------
NN = 100000          # nodes
F = 128              # feature dim (in == out)
NB = 4               # bases
NREL = 16            # relations
NCORES = 8
NS = NN // NCORES    # dst nodes per core (12500)

GROUP = 32           # dst nodes per group
GPB = 4              # groups per block
BLOCK = GROUP * GPB  # 128 dst nodes per block
NBLK = 100           # padded block count (98 real)
BPS = 4              # blocks per superblock
NSB = NBLK // BPS    # 25 superblocks

NSEG = 4             # src segments
SEG = 25000          # src rows per segment
TBL_ROWS = NN + NSEG # composite table: one zero row per segment

CS = 18              # X columns per (SB, seg): 16 base + 2 shared overflow
SEG_IDX = CS * 128   # 2304 gather slots per (SB, segment)
COLS = NSEG * CS     # 72 X columns per SB

CV = 32              # virtual mask columns per (SB, seg)
SC = NSEG * CV       # 128 virtual mask columns per SB
META_COLS = SC + SC * NB           # 640 bf16 cols per SB: ldst[SC] + w4[SC*NB]
IDX_COLS = NSEG * (SEG_IDX // 16)  # 576 int16 cols per SB

F32 = mybir.dt.float32
BF16 = mybir.dt.bfloat16
I16 = mybir.dt.int16
I32 = mybir.dt.int32
NPBF16 = ml_dtypes.bfloat16

_compiled = {}


def _build_program():
    nc = bacc.Bacc(
        "TRN2",
        target_bir_lowering=False,
        debug=False,
        enable_asserts=False,
        num_devices=NCORES,
        num_swdge_queues=4,
    )

    tbl = nc.dram_tensor("tbl", [TBL_ROWS, F], BF16, kind="ExternalInput")
    basisw = nc.dram_tensor("basisw", [NB, F, F], BF16, kind="ExternalInput")
    biasw = nc.dram_tensor("biasw", [NREL, F], F32, kind="ExternalInput")
    iota = nc.dram_tensor("iota", [128, GROUP], BF16, kind="ExternalInput")
    eidx = nc.dram_tensor("eidx", [128, NSB * IDX_COLS], I16, kind="ExternalInput")
    meta = nc.dram_tensor("meta", [128, NSB * META_COLS], BF16, kind="ExternalInput")
    gcnt = nc.dram_tensor("gcnt", [128, NSB * NSEG], I32, kind="ExternalInput")
    outT = nc.dram_tensor("outT", [NBLK, F, BLOCK], F32, kind="ExternalOutput")

    cnt_reg = nc.gpsimd.alloc_register("gcnt_reg")

    with tile.TileContext(nc) as tc:
        with (
            tc.tile_pool(name="const", bufs=1) as const,
            tc.tile_pool(name="xg", bufs=3) as xg,
            tc.tile_pool(name="idxp", bufs=2) as idxp,
            tc.tile_pool(name="metap", bufs=2) as metap,
            tc.tile_pool(name="dp", bufs=2) as dp,
            tc.tile_pool(name="msk", bufs=2) as mskp,
            tc.tile_pool(name="gt", bufs=4) as gtp,
            tc.tile_pool(name="ot", bufs=3) as otp,
            tc.tile_pool(name="psg", bufs=5, space="PSUM") as psg,
            tc.tile_pool(name="pso", bufs=2, space="PSUM") as pso,
            tc.tile_pool(name="psb", bufs=1, space="PSUM") as psb,
        ):
            # ---- constants
            iota_t = const.tile([128, GROUP], BF16)
            nc.sync.dma_start(out=iota_t[:], in_=iota[:, :])
            cnt_t = const.tile([128, NSB * NSEG], I32)
            nc.sync.dma_start(out=cnt_t[:], in_=gcnt[:, :])
            basis_t = const.tile([F, NB * F], BF16)
            for b in range(NB):
                nc.sync.dma_start(
                    out=basis_t[:, b * F : (b + 1) * F], in_=basisw[b, :, :]
                )
            bias_sb = const.tile([NREL, F], F32)
            nc.sync.dma_start(out=bias_sb[:], in_=biasw[:, :])
            ones_t = const.tile([NREL, 1], F32)
            nc.vector.memset(ones_t[:], 1.0)
            zero_t = const.tile([128, 2 * F], BF16)
            nc.vector.memset(zero_t[:], 0.0)
            bias_ps = psb.tile([F, 1], F32)
            nc.tensor.matmul(
                bias_ps[:], lhsT=bias_sb[:], rhs=ones_t[:], start=True, stop=True
            )
            bias_col = const.tile([F, 1], F32)
            nc.scalar.copy(bias_col[:], bias_ps[:])

            for sb in range(NSB):
                idx_t = idxp.tile([128, IDX_COLS], I16)
                nc.sync.dma_start(
                    out=idx_t[:], in_=eidx[:, sb * IDX_COLS : (sb + 1) * IDX_COLS]
                )
                meta_t = metap.tile([128, META_COLS], BF16)
                nc.sync.dma_start(
                    out=meta_t[:], in_=meta[:, sb * META_COLS : (sb + 1) * META_COLS]
                )

                x_t = xg.tile([128, COLS, F], BF16, tag="x")
                # the dynamic-count trim only ever skips slots in the two
                # overflow columns (counts are clamped to >= 2048); zero them
                # on the scalar engine so skipped slots never read garbage
                for s in range(NSEG):
                    nc.scalar.copy(
                        x_t[:, s * CS + 16 : s * CS + 18, :],
                        zero_t[:].rearrange("p (c f) -> p c f", c=2),
                    )

                # ---- gather: one dma_gather per src segment, one SWDGE
                # queue (= Q7 core pair) per segment -> 4-way parallel DGE.
                # The per-(SB, seg) valid-prefix length is runtime data.
                for s in range(NSEG):
                    nc.gpsimd.reg_load(
                        cnt_reg, cnt_t[0:1, sb * NSEG + s : sb * NSEG + s + 1]
                    )
                    nc.gpsimd.dma_gather(
                        out_ap=x_t[:, s * CS : (s + 1) * CS, :],
                        in_ap=tbl[s * (SEG + 1) :, :],
                        idxs_ap=idx_t[
                            :, s * (SEG_IDX // 16) : (s + 1) * (SEG_IDX // 16)
                        ],
                        num_idxs=SEG_IDX,
                        num_idxs_reg=cnt_reg,
                        elem_size=F,
                        single_packet=False,
                        queue_num=s,
                    )

                # ---- masks for the whole SB: one D op + one STT per basis
                ldst_all = meta_t[:, 0:SC]
                w4_all = meta_t[:, SC:META_COLS]
                d_t = dp.tile([128, SC * GROUP], BF16, tag="d")
                nc.vector.tensor_sub(
                    d_t[:].rearrange("p (c n) -> p c n", n=GROUP),
                    iota_t[:][:, None, :].to_broadcast([128, SC, GROUP]),
                    ldst_all[:, :, None].to_broadcast([128, SC, GROUP]),
                )
                m_t = mskp.tile([128, SC * NB * GROUP], BF16, tag="m")
                m_v = m_t[:].rearrange("p (c b n) -> p c b n", b=NB, n=GROUP)
                d_v = d_t[:].rearrange("p (c n) -> p c n", n=GROUP)
                w4_v = w4_all.rearrange("p (c b) -> p c b", b=NB)
                for bb in range(NB):
                    nc.vector.scalar_tensor_tensor(
                        out=m_v[:, :, bb, :],
                        in0=d_v,
                        scalar=0.0,
                        in1=w4_v[:, :, bb : bb + 1].to_broadcast([128, SC, GROUP]),
                        op0=mybir.AluOpType.is_equal,
                        op1=mybir.AluOpType.mult,
                    )

                gt_ps = [
                    psg.tile([F, GPB * NB * GROUP], F32, tag="g", name=f"gt{b}")
                    for b in range(BPS)
                ]

                # ---- per-column matmuls into the 4 block PSUM banks.
                # start=True arms a pending-zero for the WHOLE 2KB bank on
                # trn2, so it is issued exactly once per bank (first base
                # column of the bank at s=0); stop on the s=3 overflow matmul.
                for s in range(NSEG):
                    vs = s * CV
                    for col in range(16):
                        b, q = col // GPB, col % GPB
                        nc.tensor.matmul(
                            gt_ps[b][:, q * 128 : (q + 1) * 128],
                            lhsT=x_t[:, s * CS + col, :],
                            rhs=m_t[:, (vs + col) * 128 : (vs + col + 1) * 128],
                            start=(s == 0 and q == 0),
                            stop=False,
                            skip_group_check=True,
                        )
                    for b in range(BPS):
                        v0 = vs + 16 + 4 * b
                        nc.tensor.matmul(
                            gt_ps[b][:, 0 : 4 * 128],
                            lhsT=x_t[:, s * CS + 16 + (b // 2), :],
                            rhs=m_t[:, v0 * 128 : (v0 + 4) * 128],
                            start=False,
                            stop=(s == NSEG - 1),
                            skip_group_check=True,
                        )

                # ---- per block: basis application + bias + store
                for b in range(BPS):
                    j = sb * BPS + b
                    gt_sb = gtp.tile([F, GPB * NB * GROUP], BF16)
                    nc.scalar.copy(gt_sb[:], gt_ps[b][:])
                    ot_ps = pso.tile([F, BLOCK], F32)
                    gt_v = gt_sb[:].rearrange("p (q b n) -> p q b n", q=GPB, b=NB)
                    for bb in range(NB):
                        nc.tensor.matmul(
                            ot_ps[:].rearrange("p (q n) -> p q n", q=GPB),
                            lhsT=basis_t[:, bb * F : (bb + 1) * F],
                            rhs=gt_v[:, :, bb, :],
                            start=(bb == 0),
                            stop=(bb == NB - 1),
                        )
                    ot_sb = otp.tile([F, BLOCK], F32)
                    nc.scalar.activation(
                        ot_sb[:],
                        ot_ps[:],
                        mybir.ActivationFunctionType.Identity,
                        bias=bias_col[:],
                    )
                    nc.sync.dma_start(out=outT[j, :, :], in_=ot_sb[:])

    nc.compile()
    return nc


def _preprocess(basis_coeff, edge_val, edge_src, edge_dst):
    """Pack edges into the static (SB, segment, column) structure.
    Returns per-core (eidx [128, NSB*IDX_COLS] int16,
    meta [128, NSB*META_COLS] bf16, gcnt [128, NSB*NSEG] int32)."""
    src = np.ascontiguousarray(edge_src).ravel()
    dst = np.ascontiguousarray(edge_dst).ravel()
    val = np.ascontiguousarray(edge_val).ravel().astype(np.float32)
    rel = np.repeat(np.arange(NREL, dtype=np.int32), edge_src.shape[1])
    coeff = np.asarray(basis_coeff, dtype=np.float32)  # [NREL, NB]

    core = dst // NS
    per_core = []
    n_grp = NBLK * GPB  # 400 padded group slots (391 real)
    for c in range(NCORES):
        msel = core == c
        s_ = src[msel]
        dl = dst[msel] - c * NS
        v = val[msel]
        r = rel[msel]

        g = dl // GROUP                  # group 0..390
        w = (dl % GROUP).astype(np.float32)
        seg = s_ // SEG                  # 0..3
        lidx = (s_ % SEG + 1).astype(np.int16)  # 1..25000 (0 = zero row)

        bucket = g.astype(np.int64) * NSEG + seg
        order = np.argsort(bucket, kind="stable")
        s_, dl, v, r, g, w, seg, lidx, bucket = (
            a[order] for a in (s_, dl, v, r, g, w, seg, lidx, bucket)
        )
        cnt = np.bincount(bucket, minlength=n_grp * NSEG)
        starts = np.zeros(n_grp * NSEG + 1, dtype=np.int64)
        np.cumsum(cnt, out=starts[1:])
        pos = np.arange(len(s_)) - starts[bucket]

        j = g // GPB                     # block 0..97
        q = g % GPB
        sbi = j // BPS
        b_in = j % BPS                   # block within SB
        bis = b_in * GPB + q             # base column index, 0..15

        in128 = pos < 128
        # overflow edges: packed densely into shared column 16 (blocks 0-1)
        # or 17 (blocks 2-3), ordered by (block, q, pos); the stable sort by
        # bucket gives that order once regrouped by (SB, seg, pair)
        pair = (b_in >= 2).astype(np.int64)
        ov_idx = np.nonzero(~in128)[0]
        ov_key = (sbi[ov_idx] * NSEG + seg[ov_idx]) * 2 + pair[ov_idx]
        ov_order = np.argsort(ov_key, kind="stable")
        ov_sorted = ov_idx[ov_order]
        ov_key_sorted = ov_key[ov_order]
        ov_cnt = np.bincount(ov_key_sorted, minlength=NSB * NSEG * 2)
        assert ov_cnt.max() <= 128, f"overflow column exceeded: {ov_cnt.max()}"
        ov_starts = np.zeros(NSB * NSEG * 2 + 1, dtype=np.int64)
        np.cumsum(ov_cnt, out=ov_starts[1:])
        ovpos = np.zeros(len(s_), dtype=np.int64)
        ovpos[ov_sorted] = np.arange(len(ov_sorted)) - ov_starts[ov_key_sorted]

        col = np.where(in128, bis, 16 + pair)          # physical col 0..17
        part = np.where(in128, pos, ovpos)
        vc = np.where(in128, bis, 16 + 4 * b_in + q)   # virtual col 0..31
        gpos = col * 128 + part

        # ---- index array: valid prefix + trailing -1; per-(SB,seg) counts
        idx_flat = np.zeros((NSB, NSEG, SEG_IDX), dtype=np.int16)
        idx_flat[sbi, seg, gpos] = lidx
        maxg = np.full((NSB, NSEG), -1, dtype=np.int64)
        np.maximum.at(maxg, (sbi, seg), gpos)
        # valid-prefix length, clamped so trimming stays within the two
        # overflow columns (base columns are always fully gathered)
        maxg = np.maximum(maxg, 16 * 128 - 1)
        trail = np.arange(SEG_IDX)[None, None, :] > maxg[:, :, None]
        idx_flat[trail] = -1
        counts = (maxg + 1).astype(np.int32)

        # wrap: position i = s16*16 + p16 -> [16, SEG_IDX//16], tiled to 128
        wrapped = idx_flat.reshape(NSB, NSEG, SEG_IDX // 16, 16).transpose(0, 1, 3, 2)
        wrapped = np.broadcast_to(
            wrapped[:, :, None, :, :], (NSB, NSEG, 8, 16, SEG_IDX // 16)
        ).reshape(NSB, NSEG, 128, SEG_IDX // 16)
        eidx_c = np.ascontiguousarray(
            wrapped.transpose(2, 0, 1, 3).reshape(128, NSB * IDX_COLS)
        )

        # ---- meta per SB: [ldst: SC][w4: SC*NB], sc = seg*CV + vc, bf16
        scix = seg * CV + vc
        mldst = np.zeros((NSB, 128, SC), dtype=np.float32)
        mw4 = np.zeros((NSB, 128, SC, NB), dtype=np.float32)
        mldst[sbi, part, scix] = w
        mw4[sbi, part, scix] = v[:, None] * coeff[r]
        meta_c = np.concatenate(
            [mldst, mw4.reshape(NSB, 128, SC * NB)], axis=2
        )  # [NSB, 128, META_COLS]
        meta_c = np.ascontiguousarray(
            meta_c.transpose(1, 0, 2).reshape(128, NSB * META_COLS)
        ).astype(NPBF16)

        gcnt_c = np.ascontiguousarray(
            np.broadcast_to(
                counts.reshape(1, NSB * NSEG), (128, NSB * NSEG)
            )
        ).astype(np.int32)
        per_core.append((eidx_c, meta_c, gcnt_c))
    return per_core


def _build_table(inp):
    tbl = np.zeros((TBL_ROWS, F), dtype=np.float32)
    for s in range(NSEG):
        tbl[s * (SEG + 1) + 1 : (s + 1) * (SEG + 1)] = inp[s * SEG : (s + 1) * SEG]
    return tbl.astype(NPBF16)


def kernel(inp, basis_weights, basis_coeff, bias, edge_val, edge_src, edge_dst):
    inp = np.ascontiguousarray(np.asarray(inp, dtype=np.float32))
    basis_weights = np.ascontiguousarray(np.asarray(basis_weights, dtype=np.float32))
    basis_coeff = np.asarray(basis_coeff, dtype=np.float32)
    bias = np.ascontiguousarray(np.asarray(bias, dtype=np.float32))

    if "nc" not in _compiled:
        _compiled["nc"] = _build_program()
    nc = _compiled["nc"]

    per_core = _preprocess(basis_coeff, edge_val, edge_src, edge_dst)
    tbl = _build_table(inp)
    iota_np = np.ascontiguousarray(
        np.arange(GROUP, dtype=np.float32)[None, :].repeat(128, 0)
    ).astype(NPBF16)
    basis_bf = basis_weights.astype(NPBF16)

    in_maps = []
    for c in range(NCORES):
        eidx_c, meta_c, gcnt_c = per_core[c]
        in_maps.append(
            {
                "tbl": tbl,
                "basisw": basis_bf,
                "biasw": bias,
                "iota": iota_np,
                "eidx": eidx_c,
                "meta": meta_c,
                "gcnt": gcnt_c,
            }
        )

    res = run_bass_kernel_spmd(nc, in_maps, list(range(NCORES)))
    _compiled["last_results"] = res

    out = np.empty((NN, F), dtype=np.float32)
    for c in range(NCORES):
        oT = res.results[c]["outT"]  # [NBLK, F, BLOCK]
        rows = oT.transpose(0, 2, 1).reshape(NBLK * BLOCK, F)[:NS]
        out[c * NS : (c + 1) * NS] = rows
    return out


# revision 11
# speedup vs baseline: 1.0823x; 1.0823x over previous
"""Trainium2 Bass kernel for an R-GCN-style GCN layer (basis decomposition).

Reference computation (per relation r, with W_r = sum_b coeff[r,b] * basis[b]):
    out = sum_r segment_sum(inp[src_r] * val_r, dst_r) @ W_r + sum_r bias[r]

Algebraic restructure (4 basis accumulators instead of 16 relation matmuls):
    out[d] = sum_b G_b[d] @ basis[b] + bias_sum
    G_b[d] = sum_{edges e: dst_e = d} (coeff[r_e, b] * val_e) * inp[src_e]

Distribution: output nodes are sharded 8 ways (12500 rows/core); every core
holds the full gather table in its own HBM -> no cross-core communication.

Key performance structure:
  - The per-edge feature gather runs as dma_gather on 4 SWDGE queues
    (queue q is served by GPSIMD Q7 core pair (2q, 2q+1)), one queue per
    src segment, so descriptor generation runs 4-way parallel.
  - Whole datapath in bf16: gather table, masks, matmuls (PSUM stays fp32).
  - 18 X columns per (SB, seg): 16 base bucket columns (one per
    (block-in-SB, 32-dst-group)) plus 2 shared overflow columns (blocks
    0-1 -> col 16, blocks 2-3 -> col 17, edges packed densely); overflow
    matmul masks span the whole 512-wide PSUM bank of their block.
  - Per-(SB, seg) valid-index counts are loaded into a Pool register at
    runtime (per-core data) and passed as num_idxs_reg, so dma_gather
    skips all trailing padding slots (the valid slots form a prefix,
    trailing slots hold idx = -1).
  - Masks are built with 5 DVE ops per SB over 4x32 "virtual columns"
    (16 base + 16 overflow quarters per segment):
        D[p, sc, n]    = iota[n] - ldst[p, sc]          (one tensor_sub)
        M[p, sc, b, n] = (D == 0) * w4[p, sc, b]        (one STT per basis)
    with ldst/w4 packed per virtual column on the host.

Per-core shapes are identical across cores (SPMD); only data differs.
Output is produced transposed per block ([fout, node]) and reassembled on host.
"""
import os
import sys

for _p in ("/opt/trn_rl_repo", "/root/.axon_site/_ro/trn_rl_repo"):
    if os.path.isdir(_p) and _p not in sys.path:
        sys.path.insert(0, _p)

import ml_dtypes
import numpy as np

import concourse.bass as bass
import concourse.tile as tile
from concourse import bacc, mybir
from concourse.bass_utils import run_bass_kernel_spmd

# ---------------- problem constants (hardcoded from spec) ----------------
NN = 100000          # nodes
F = 128              # feature dim (in == out)
NB = 4               # bases
NREL = 16            # relations
NCORES = 8
NS = NN // NCORES    # dst nodes per core (12500)

GROUP = 32           # dst nodes per group
GPB = 4              # groups per block
BLOCK = GROUP * GPB  # 128 dst nodes per block
NBLK = 100           # padded block count (98 real)
BPS = 4              # blocks per superblock
NSB = NBLK // BPS    # 25 superblocks

NSEG = 4             # src segments
SEG = 25000          # src rows per segment
TBL_ROWS = NN + NSEG # composite table: one zero row per segment

CS = 18              # X columns per (SB, seg): 16 base + 2 shared overflow
SEG_IDX = CS * 128   # 2304 gather slots per (SB, segment)
COLS = NSEG * CS     # 72 X columns per SB

CV = 32              # virtual mask columns per (SB, seg)
SC = NSEG * CV       # 128 virtual mask columns per SB
META_COLS = SC + SC * NB           # 640 bf16 cols per SB: ldst[SC] + w4[SC*NB]
IDX_COLS = NSEG * (SEG_IDX // 16)  # 576 int16 cols per SB

F32 = mybir.dt.float32
BF16 = mybir.dt.bfloat16
I16 = mybir.dt.int16
I32 = mybir.dt.int32
NPBF16 = ml_dtypes.bfloat16

_compiled = {}


def _build_program():
    nc = bacc.Bacc(
        "TRN2",
        target_bir_lowering=False,
        debug=False,
        enable_asserts=False,
        num_devices=NCORES,
        num_swdge_queues=4,
    )

    tbl = nc.dram_tensor("tbl", [TBL_ROWS, F], BF16, kind="ExternalInput")
    basisw = nc.dram_tensor("basisw", [NB, F, F], BF16, kind="ExternalInput")
    biasw = nc.dram_tensor("biasw", [NREL, F], F32, kind="ExternalInput")
    iota = nc.dram_tensor("iota", [128, GROUP], BF16, kind="ExternalInput")
    eidx = nc.dram_tensor("eidx", [128, NSB * IDX_COLS], I16, kind="ExternalInput")
    meta = nc.dram_tensor("meta", [128, NSB * META_COLS], BF16, kind="ExternalInput")
    gcnt = nc.dram_tensor("gcnt", [128, NSB * NSEG], I32, kind="ExternalInput")
    outT = nc.dram_tensor("outT", [NBLK, F, BLOCK], F32, kind="ExternalOutput")

    cnt_reg = nc.gpsimd.alloc_register("gcnt_reg")

    with tile.TileContext(nc) as tc:
        with (
            tc.tile_pool(name="const", bufs=1) as const,
            tc.tile_pool(name="xg", bufs=3) as xg,
            tc.tile_pool(name="idxp", bufs=2) as idxp,
            tc.tile_pool(name="metap", bufs=2) as metap,
            tc.tile_pool(name="dp", bufs=4) as dp,
            tc.tile_pool(name="msk", bufs=8) as mskp,
            tc.tile_pool(name="gt", bufs=4) as gtp,
            tc.tile_pool(name="ot", bufs=3) as otp,
            tc.tile_pool(name="psg", bufs=5, space="PSUM") as psg,
            tc.tile_pool(name="pso", bufs=2, space="PSUM") as pso,
            tc.tile_pool(name="psb", bufs=1, space="PSUM") as psb,
        ):
            # ---- constants
            iota_t = const.tile([128, GROUP], BF16)
            nc.sync.dma_start(out=iota_t[:], in_=iota[:, :])
            cnt_t = const.tile([128, NSB * NSEG], I32)
            nc.sync.dma_start(out=cnt_t[:], in_=gcnt[:, :])
            basis_t = const.tile([F, NB * F], BF16)
            for b in range(NB):
                nc.sync.dma_start(
                    out=basis_t[:, b * F : (b + 1) * F], in_=basisw[b, :, :]
                )
            bias_sb = const.tile([NREL, F], F32)
            nc.sync.dma_start(out=bias_sb[:], in_=biasw[:, :])
            ones_t = const.tile([NREL, 1], F32)
            nc.vector.memset(ones_t[:], 1.0)
            zero_t = const.tile([128, 2 * F], BF16)
            nc.vector.memset(zero_t[:], 0.0)
            bias_ps = psb.tile([F, 1], F32)
            nc.tensor.matmul(
                bias_ps[:], lhsT=bias_sb[:], rhs=ones_t[:], start=True, stop=True
            )
            bias_col = const.tile([F, 1], F32)
            nc.scalar.copy(bias_col[:], bias_ps[:])

            for sb in range(NSB):
                idx_t = idxp.tile([128, IDX_COLS], I16)
                nc.sync.dma_start(
                    out=idx_t[:], in_=eidx[:, sb * IDX_COLS : (sb + 1) * IDX_COLS]
                )
                meta_t = metap.tile([128, META_COLS], BF16)
                nc.sync.dma_start(
                    out=meta_t[:], in_=meta[:, sb * META_COLS : (sb + 1) * META_COLS]
                )

                x_t = xg.tile([128, COLS, F], BF16, tag="x")
                # the dynamic-count trim only ever skips slots in the two
                # overflow columns (counts are clamped to >= 2048). Zero them
                # on first use of each ring buffer so skipped slots never read
                # NaN bit patterns; on later reuse the stale contents are old
                # gathered rows (finite) and the zero masks annihilate them.
                # (CoreSim poisons recycled tiles, so sim runs zero every SB.)
                if sb < 3 or os.environ.get("GCN_SIM_ZERO"):
                    for s in range(NSEG):
                        nc.scalar.copy(
                            x_t[:, s * CS + 16 : s * CS + 18, :],
                            zero_t[:].rearrange("p (c f) -> p c f", c=2),
                        )

                # ---- gather: one dma_gather per src segment, one SWDGE
                # queue (= Q7 core pair) per segment -> 4-way parallel DGE.
                # The per-(SB, seg) valid-prefix length is runtime data.
                for s in range(NSEG):
                    nc.gpsimd.reg_load(
                        cnt_reg, cnt_t[0:1, sb * NSEG + s : sb * NSEG + s + 1]
                    )
                    nc.gpsimd.dma_gather(
                        out_ap=x_t[:, s * CS : (s + 1) * CS, :],
                        in_ap=tbl[s * (SEG + 1) :, :],
                        idxs_ap=idx_t[
                            :, s * (SEG_IDX // 16) : (s + 1) * (SEG_IDX // 16)
                        ],
                        num_idxs=SEG_IDX,
                        num_idxs_reg=cnt_reg,
                        elem_size=F,
                        single_packet=False,
                        queue_num=s,
                    )

                gt_ps = [
                    psg.tile([F, GPB * NB * GROUP], F32, tag="g", name=f"gt{b}")
                    for b in range(BPS)
                ]

                # ---- per segment: masks (one D op + one STT per basis over
                # this segment's 32 virtual columns), then the matmuls.
                # start=True arms a pending-zero for the WHOLE 2KB bank on
                # trn2, so it is issued exactly once per bank (first base
                # column of the bank at s=0); stop on the s=3 overflow matmul.
                for s in range(NSEG):
                    ldst_s = meta_t[:, s * CV : (s + 1) * CV]
                    w4_s = meta_t[:, SC + s * CV * NB : SC + (s + 1) * CV * NB]
                    d_t = dp.tile([128, CV * GROUP], BF16, tag="d")
                    nc.vector.tensor_sub(
                        d_t[:].rearrange("p (c n) -> p c n", n=GROUP),
                        iota_t[:][:, None, :].to_broadcast([128, CV, GROUP]),
                        ldst_s[:, :, None].to_broadcast([128, CV, GROUP]),
                    )
                    m_t = mskp.tile([128, CV * NB * GROUP], BF16, tag="m")
                    m_v = m_t[:].rearrange("p (c b n) -> p c b n", b=NB, n=GROUP)
                    d_v = d_t[:].rearrange("p (c n) -> p c n", n=GROUP)
                    w4_v = w4_s.rearrange("p (c b) -> p c b", b=NB)
                    for bb in range(NB):
                        nc.vector.scalar_tensor_tensor(
                            out=m_v[:, :, bb, :],
                            in0=d_v,
                            scalar=0.0,
                            in1=w4_v[:, :, bb : bb + 1].to_broadcast(
                                [128, CV, GROUP]
                            ),
                            op0=mybir.AluOpType.is_equal,
                            op1=mybir.AluOpType.mult,
                        )

                    for col in range(16):
                        b, q = col // GPB, col % GPB
                        nc.tensor.matmul(
                            gt_ps[b][:, q * 128 : (q + 1) * 128],
                            lhsT=x_t[:, s * CS + col, :],
                            rhs=m_t[:, col * 128 : (col + 1) * 128],
                            start=(s == 0 and q == 0),
                            stop=False,
                            skip_group_check=True,
                        )
                    for b in range(BPS):
                        v0 = 16 + 4 * b
                        nc.tensor.matmul(
                            gt_ps[b][:, 0 : 4 * 128],
                            lhsT=x_t[:, s * CS + 16 + (b // 2), :],
                            rhs=m_t[:, v0 * 128 : (v0 + 4) * 128],
                            start=False,
                            stop=(s == NSEG - 1),
                            skip_group_check=True,
                        )

                # ---- per block: basis application + bias + store
                for b in range(BPS):
                    j = sb * BPS + b
                    gt_sb = gtp.tile([F, GPB * NB * GROUP], BF16)
                    nc.scalar.copy(gt_sb[:], gt_ps[b][:])
                    ot_ps = pso.tile([F, BLOCK], F32)
                    gt_v = gt_sb[:].rearrange("p (q b n) -> p q b n", q=GPB, b=NB)
                    for bb in range(NB):
                        nc.tensor.matmul(
                            ot_ps[:].rearrange("p (q n) -> p q n", q=GPB),
                            lhsT=basis_t[:, bb * F : (bb + 1) * F],
                            rhs=gt_v[:, :, bb, :],
                            start=(bb == 0),
                            stop=(bb == NB - 1),
                        )
                    ot_sb = otp.tile([F, BLOCK], F32)
                    nc.scalar.activation(
                        ot_sb[:],
                        ot_ps[:],
                        mybir.ActivationFunctionType.Identity,
                        bias=bias_col[:],
                    )
                    nc.sync.dma_start(out=outT[j, :, :], in_=ot_sb[:])

    nc.compile()
    return nc


def _preprocess(basis_coeff, edge_val, edge_src, edge_dst):
    """Pack edges into the static (SB, segment, column) structure.
    Returns per-core (eidx [128, NSB*IDX_COLS] int16,
    meta [128, NSB*META_COLS] bf16, gcnt [128, NSB*NSEG] int32)."""
    src = np.ascontiguousarray(edge_src).ravel()
    dst = np.ascontiguousarray(edge_dst).ravel()
    val = np.ascontiguousarray(edge_val).ravel().astype(np.float32)
    rel = np.repeat(np.arange(NREL, dtype=np.int32), edge_src.shape[1])
    coeff = np.asarray(basis_coeff, dtype=np.float32)  # [NREL, NB]

    core = dst // NS
    per_core = []
    n_grp = NBLK * GPB  # 400 padded group slots (391 real)
    for c in range(NCORES):
        msel = core == c
        s_ = src[msel]
        dl = dst[msel] - c * NS
        v = val[msel]
        r = rel[msel]

        g = dl // GROUP                  # group 0..390
        w = (dl % GROUP).astype(np.float32)
        seg = s_ // SEG                  # 0..3
        lidx = (s_ % SEG + 1).astype(np.int16)  # 1..25000 (0 = zero row)

        bucket = g.astype(np.int64) * NSEG + seg
        order = np.argsort(bucket, kind="stable")
        s_, dl, v, r, g, w, seg, lidx, bucket = (
            a[order] for a in (s_, dl, v, r, g, w, seg, lidx, bucket)
        )
        cnt = np.bincount(bucket, minlength=n_grp * NSEG)
        starts = np.zeros(n_grp * NSEG + 1, dtype=np.int64)
        np.cumsum(cnt, out=starts[1:])
        pos = np.arange(len(s_)) - starts[bucket]

        j = g // GPB                     # block 0..97
        q = g % GPB
        sbi = j // BPS
        b_in = j % BPS                   # block within SB
        bis = b_in * GPB + q             # base column index, 0..15

        in128 = pos < 128
        # overflow edges: packed densely into shared column 16 (blocks 0-1)
        # or 17 (blocks 2-3), ordered by (block, q, pos); the stable sort by
        # bucket gives that order once regrouped by (SB, seg, pair)
        pair = (b_in >= 2).astype(np.int64)
        ov_idx = np.nonzero(~in128)[0]
        ov_key = (sbi[ov_idx] * NSEG + seg[ov_idx]) * 2 + pair[ov_idx]
        ov_order = np.argsort(ov_key, kind="stable")
        ov_sorted = ov_idx[ov_order]
        ov_key_sorted = ov_key[ov_order]
        ov_cnt = np.bincount(ov_key_sorted, minlength=NSB * NSEG * 2)
        assert ov_cnt.max() <= 128, f"overflow column exceeded: {ov_cnt.max()}"
        ov_starts = np.zeros(NSB * NSEG * 2 + 1, dtype=np.int64)
        np.cumsum(ov_cnt, out=ov_starts[1:])
        ovpos = np.zeros(len(s_), dtype=np.int64)
        ovpos[ov_sorted] = np.arange(len(ov_sorted)) - ov_starts[ov_key_sorted]

        col = np.where(in128, bis, 16 + pair)          # physical col 0..17
        part = np.where(in128, pos, ovpos)
        vc = np.where(in128, bis, 16 + 4 * b_in + q)   # virtual col 0..31
        gpos = col * 128 + part

        # ---- index array: valid prefix + trailing -1; per-(SB,seg) counts
        idx_flat = np.zeros((NSB, NSEG, SEG_IDX), dtype=np.int16)
        idx_flat[sbi, seg, gpos] = lidx
        maxg = np.full((NSB, NSEG), -1, dtype=np.int64)
        np.maximum.at(maxg, (sbi, seg), gpos)
        # valid-prefix length, clamped so trimming stays within the two
        # overflow columns (base columns are always fully gathered)
        maxg = np.maximum(maxg, 16 * 128 - 1)
        trail = np.arange(SEG_IDX)[None, None, :] > maxg[:, :, None]
        idx_flat[trail] = -1
        counts = (maxg + 1).astype(np.int32)

        # wrap: position i = s16*16 + p16 -> [16, SEG_IDX//16], tiled to 128
        wrapped = idx_flat.reshape(NSB, NSEG, SEG_IDX // 16, 16).transpose(0, 1, 3, 2)
        wrapped = np.broadcast_to(
            wrapped[:, :, None, :, :], (NSB, NSEG, 8, 16, SEG_IDX // 16)
        ).reshape(NSB, NSEG, 128, SEG_IDX // 16)
        eidx_c = np.ascontiguousarray(
            wrapped.transpose(2, 0, 1, 3).reshape(128, NSB * IDX_COLS)
        )

        # ---- meta per SB: [ldst: SC][w4: SC*NB], sc = seg*CV + vc, bf16
        scix = seg * CV + vc
        mldst = np.zeros((NSB, 128, SC), dtype=np.float32)
        mw4 = np.zeros((NSB, 128, SC, NB), dtype=np.float32)
        mldst[sbi, part, scix] = w
        mw4[sbi, part, scix] = v[:, None] * coeff[r]
        meta_c = np.concatenate(
            [mldst, mw4.reshape(NSB, 128, SC * NB)], axis=2
        )  # [NSB, 128, META_COLS]
        meta_c = np.ascontiguousarray(
            meta_c.transpose(1, 0, 2).reshape(128, NSB * META_COLS)
        ).astype(NPBF16)

        gcnt_c = np.ascontiguousarray(
            np.broadcast_to(
                counts.reshape(1, NSB * NSEG), (128, NSB * NSEG)
            )
        ).astype(np.int32)
        per_core.append((eidx_c, meta_c, gcnt_c))
    return per_core


def _build_table(inp):
    tbl = np.zeros((TBL_ROWS, F), dtype=np.float32)
    for s in range(NSEG):
        tbl[s * (SEG + 1) + 1 : (s + 1) * (SEG + 1)] = inp[s * SEG : (s + 1) * SEG]
    return tbl.astype(NPBF16)


def kernel(inp, basis_weights, basis_coeff, bias, edge_val, edge_src, edge_dst):
    inp = np.ascontiguousarray(np.asarray(inp, dtype=np.float32))
    basis_weights = np.ascontiguousarray(np.asarray(basis_weights, dtype=np.float32))
    basis_coeff = np.asarray(basis_coeff, dtype=np.float32)
    bias = np.ascontiguousarray(np.asarray(bias, dtype=np.float32))

    if "nc" not in _compiled:
        _compiled["nc"] = _build_program()
    nc = _compiled["nc"]

    per_core = _preprocess(basis_coeff, edge_val, edge_src, edge_dst)
    tbl = _build_table(inp)
    iota_np = np.ascontiguousarray(
        np.arange(GROUP, dtype=np.float32)[None, :].repeat(128, 0)
    ).astype(NPBF16)
    basis_bf = basis_weights.astype(NPBF16)

    in_maps = []
    for c in range(NCORES):
        eidx_c, meta_c, gcnt_c = per_core[c]
        in_maps.append(
            {
                "tbl": tbl,
                "basisw": basis_bf,
                "biasw": bias,
                "iota": iota_np,
                "eidx": eidx_c,
                "meta": meta_c,
                "gcnt": gcnt_c,
            }
        )

    res = run_bass_kernel_spmd(nc, in_maps, list(range(NCORES)))
    _compiled["last_results"] = res

    out = np.empty((NN, F), dtype=np.float32)
    for c in range(NCORES):
        oT = res.results[c]["outT"]  # [NBLK, F, BLOCK]
        rows = oT.transpose(0, 2, 1).reshape(NBLK * BLOCK, F)[:NS]
        out[c * NS : (c + 1) * NS] = rows
    return out


# revision 15
# speedup vs baseline: 1.1020x; 1.0183x over previous
"""Trainium2 Bass kernel for an R-GCN-style GCN layer (basis decomposition).

Reference computation (per relation r, with W_r = sum_b coeff[r,b] * basis[b]):
    out = sum_r segment_sum(inp[src_r] * val_r, dst_r) @ W_r + sum_r bias[r]

Algebraic restructure (4 basis accumulators instead of 16 relation matmuls):
    out[d] = sum_b G_b[d] @ basis[b] + bias_sum
    G_b[d] = sum_{edges e: dst_e = d} (coeff[r_e, b] * val_e) * inp[src_e]

Distribution: output nodes are sharded 8 ways (12500 rows/core); every core
holds the full gather table in its own HBM -> no cross-core communication.

Key performance structure:
  - The per-edge feature gather runs as dma_gather on 4 SWDGE queues
    (queue q is served by GPSIMD Q7 core pair (2q, 2q+1)), one queue per
    src segment, so descriptor generation runs 4-way parallel.
  - Whole datapath in bf16: gather table, masks, matmuls (PSUM stays fp32).
  - 18 X columns per (SB, seg): 16 base bucket columns (one per
    (block-in-SB, 32-dst-group)) plus 2 shared overflow columns (blocks
    0-1 -> col 16, blocks 2-3 -> col 17, edges packed densely); overflow
    matmul masks span the whole 512-wide PSUM bank of their block.
  - Per-(SB, seg) valid-index counts are loaded into a Pool register at
    runtime (per-core data) and passed as num_idxs_reg, so dma_gather
    skips all trailing padding slots (the valid slots form a prefix,
    trailing slots hold idx = -1).
  - Masks are built with 5 DVE ops per SB over 4x32 "virtual columns"
    (16 base + 16 overflow quarters per segment):
        D[p, sc, n]    = iota[n] - ldst[p, sc]          (one tensor_sub)
        M[p, sc, b, n] = (D == 0) * w4[p, sc, b]        (one STT per basis)
    with ldst/w4 packed per virtual column on the host.

Per-core shapes are identical across cores (SPMD); only data differs.
Output is produced transposed per block ([fout, node]) and reassembled on host.
"""
import os
import sys

for _p in ("/opt/trn_rl_repo", "/root/.axon_site/_ro/trn_rl_repo"):
    if os.path.isdir(_p) and _p not in sys.path:
        sys.path.insert(0, _p)

import ml_dtypes
import numpy as np

import concourse.bass as bass
import concourse.tile as tile
from concourse import bacc, mybir
from concourse.bass_utils import run_bass_kernel_spmd

# ---------------- problem constants (hardcoded from spec) ----------------
NN = 100000          # nodes
F = 128              # feature dim (in == out)
NB = 4               # bases
NREL = 16            # relations
NCORES = 8
NS = NN // NCORES    # dst nodes per core (12500)

GROUP = 32           # dst nodes per group
GPB = 4              # groups per block
BLOCK = GROUP * GPB  # 128 dst nodes per block
NBLK = 100           # padded block count (98 real)
BPS = 4              # blocks per superblock
NSB = NBLK // BPS    # 25 superblocks

NSEG = 4             # src segments
SEG = 25000          # src rows per segment
TBL_ROWS = NN + NSEG # composite table: one zero row per segment

CS = 18              # X columns per (SB, seg): 16 base + 2 shared overflow
SEG_IDX = CS * 128   # 2304 gather slots per (SB, segment)
COLS = NSEG * CS     # 72 X columns per SB

CV = 32              # virtual mask columns per (SB, seg)
SC = NSEG * CV       # 128 virtual mask columns per SB
META_COLS = SC + SC * NB           # 640 bf16 cols per SB: ldst[SC] + w4[SC*NB]
IDX_COLS = NSEG * (SEG_IDX // 16)  # 576 int16 cols per SB

F32 = mybir.dt.float32
BF16 = mybir.dt.bfloat16
I16 = mybir.dt.int16
I32 = mybir.dt.int32
NPBF16 = ml_dtypes.bfloat16

_compiled = {}


def _build_program():
    nc = bacc.Bacc(
        "TRN2",
        target_bir_lowering=False,
        debug=False,
        enable_asserts=False,
        num_devices=NCORES,
        num_swdge_queues=4,
    )

    tbl = nc.dram_tensor("tbl", [TBL_ROWS, F], BF16, kind="ExternalInput")
    basisw = nc.dram_tensor("basisw", [NB, F, F], BF16, kind="ExternalInput")
    biasw = nc.dram_tensor("biasw", [NREL, F], F32, kind="ExternalInput")
    iota = nc.dram_tensor("iota", [128, GROUP], BF16, kind="ExternalInput")
    eidx = nc.dram_tensor("eidx", [128, NSB * IDX_COLS], I16, kind="ExternalInput")
    meta = nc.dram_tensor("meta", [128, NSB * META_COLS], BF16, kind="ExternalInput")
    gcnt = nc.dram_tensor("gcnt", [128, NSB * NSEG], I32, kind="ExternalInput")
    outT = nc.dram_tensor("outT", [NSB, F, BPS * BLOCK], F32, kind="ExternalOutput")

    cnt_reg = nc.gpsimd.alloc_register("gcnt_reg")

    with tile.TileContext(nc) as tc:
        with (
            tc.tile_pool(name="const", bufs=1) as const,
            tc.tile_pool(name="xg", bufs=4) as xg,
            tc.tile_pool(name="idxp", bufs=2) as idxp,
            tc.tile_pool(name="metap", bufs=2) as metap,
            tc.tile_pool(name="dp", bufs=6) as dp,
            tc.tile_pool(name="msk", bufs=8) as mskp,
            tc.tile_pool(name="gt", bufs=4) as gtp,
            tc.tile_pool(name="ot", bufs=3) as otp,
            tc.tile_pool(name="psg", bufs=5, space="PSUM") as psg,
            tc.tile_pool(name="pso", bufs=2, space="PSUM") as pso,
            tc.tile_pool(name="psb", bufs=1, space="PSUM") as psb,
        ):
            # ---- constants
            iota_t = const.tile([128, GROUP], BF16)
            nc.sync.dma_start(out=iota_t[:], in_=iota[:, :])
            cnt_t = const.tile([128, NSB * NSEG], I32)
            nc.sync.dma_start(out=cnt_t[:], in_=gcnt[:, :])
            basis_t = const.tile([F, NB * F], BF16)
            for b in range(NB):
                nc.sync.dma_start(
                    out=basis_t[:, b * F : (b + 1) * F], in_=basisw[b, :, :]
                )
            bias_sb = const.tile([NREL, F], F32)
            nc.sync.dma_start(out=bias_sb[:], in_=biasw[:, :])
            ones_t = const.tile([NREL, 1], F32)
            nc.vector.memset(ones_t[:], 1.0)
            zero_t = const.tile([128, 2 * F], BF16)
            nc.vector.memset(zero_t[:], 0.0)
            bias_ps = psb.tile([F, 1], F32)
            nc.tensor.matmul(
                bias_ps[:], lhsT=bias_sb[:], rhs=ones_t[:], start=True, stop=True
            )
            bias_col = const.tile([F, 1], F32)
            nc.scalar.copy(bias_col[:], bias_ps[:])

            for sb in range(NSB):
                idx_t = idxp.tile([128, IDX_COLS], I16)
                nc.sync.dma_start(
                    out=idx_t[:], in_=eidx[:, sb * IDX_COLS : (sb + 1) * IDX_COLS]
                )
                meta_t = metap.tile([128, META_COLS], BF16)
                nc.sync.dma_start(
                    out=meta_t[:], in_=meta[:, sb * META_COLS : (sb + 1) * META_COLS]
                )

                x_t = xg.tile([128, COLS, F], BF16, tag="x")
                # the dynamic-count trim only ever skips slots in the two
                # overflow columns (counts are clamped to >= 2048). Zero them
                # on first use of each ring buffer so skipped slots never read
                # NaN bit patterns; on later reuse the stale contents are old
                # gathered rows (finite) and the zero masks annihilate them.
                # (CoreSim poisons recycled tiles, so sim runs zero every SB.)
                if sb < 3 or os.environ.get("GCN_SIM_ZERO"):
                    for s in range(NSEG):
                        nc.scalar.copy(
                            x_t[:, s * CS + 16 : s * CS + 18, :],
                            zero_t[:].rearrange("p (c f) -> p c f", c=2),
                        )

                # ---- gather: one dma_gather per src segment, one SWDGE
                # queue (= Q7 core pair) per segment -> 4-way parallel DGE.
                # The per-(SB, seg) valid-prefix length is runtime data.
                for s in range(NSEG):
                    nc.gpsimd.reg_load(
                        cnt_reg, cnt_t[0:1, sb * NSEG + s : sb * NSEG + s + 1]
                    )
                    nc.gpsimd.dma_gather(
                        out_ap=x_t[:, s * CS : (s + 1) * CS, :],
                        in_ap=tbl[s * (SEG + 1) :, :],
                        idxs_ap=idx_t[
                            :, s * (SEG_IDX // 16) : (s + 1) * (SEG_IDX // 16)
                        ],
                        num_idxs=SEG_IDX,
                        num_idxs_reg=cnt_reg,
                        elem_size=F,
                        single_packet=False,
                        queue_num=s,
                    )

                gt_ps = [
                    psg.tile([F, GPB * NB * GROUP], F32, tag="g", name=f"gt{b}")
                    for b in range(BPS)
                ]

                # ---- per segment: masks (one D op + one STT per basis over
                # this segment's 32 virtual columns), then the matmuls.
                # start=True arms a pending-zero for the WHOLE 2KB bank on
                # trn2, so it is issued exactly once per bank (first base
                # column of the bank at s=0); stop on the s=3 overflow matmul.
                for s in range(NSEG):
                    ldst_s = meta_t[:, s * CV : (s + 1) * CV]
                    w4_s = meta_t[:, SC + s * CV * NB : SC + (s + 1) * CV * NB]
                    d_t = dp.tile([128, CV * GROUP], BF16, tag="d")
                    nc.vector.tensor_sub(
                        d_t[:].rearrange("p (c n) -> p c n", n=GROUP),
                        iota_t[:][:, None, :].to_broadcast([128, CV, GROUP]),
                        ldst_s[:, :, None].to_broadcast([128, CV, GROUP]),
                    )
                    m_t = mskp.tile([128, CV * NB * GROUP], BF16, tag="m")
                    m_v = m_t[:].rearrange("p (c b n) -> p c b n", b=NB, n=GROUP)
                    d_v = d_t[:].rearrange("p (c n) -> p c n", n=GROUP)
                    w4_v = w4_s.rearrange("p (c b) -> p c b", b=NB)
                    for bb in range(NB):
                        nc.vector.scalar_tensor_tensor(
                            out=m_v[:, :, bb, :],
                            in0=d_v,
                            scalar=0.0,
                            in1=w4_v[:, :, bb : bb + 1].to_broadcast(
                                [128, CV, GROUP]
                            ),
                            op0=mybir.AluOpType.is_equal,
                            op1=mybir.AluOpType.mult,
                        )

                    for col in range(16):
                        b, q = col // GPB, col % GPB
                        nc.tensor.matmul(
                            gt_ps[b][:, q * 128 : (q + 1) * 128],
                            lhsT=x_t[:, s * CS + col, :],
                            rhs=m_t[:, col * 128 : (col + 1) * 128],
                            start=(s == 0 and q == 0),
                            stop=False,
                            skip_group_check=True,
                        )
                    for b in range(BPS):
                        v0 = 16 + 4 * b
                        nc.tensor.matmul(
                            gt_ps[b][:, 0 : 4 * 128],
                            lhsT=x_t[:, s * CS + 16 + (b // 2), :],
                            rhs=m_t[:, v0 * 128 : (v0 + 4) * 128],
                            start=False,
                            stop=(s == NSEG - 1),
                            skip_group_check=True,
                        )

                # ---- basis application for the whole SB into one shared
                # PSUM bank ([F, 4 blocks * 128 nodes]); bb-outer order so
                # each basis matrix stays loaded for 4 consecutive matmuls.
                gt_sbs = []
                for b in range(BPS):
                    gt_sb = gtp.tile([F, GPB * NB * GROUP], BF16)
                    nc.scalar.copy(gt_sb[:], gt_ps[b][:])
                    gt_sbs.append(
                        gt_sb[:].rearrange("p (q b n) -> p q b n", q=GPB, b=NB)
                    )
                ot_ps = pso.tile([F, BPS * BLOCK], F32)
                for bb in range(NB):
                    for b in range(BPS):
                        nc.tensor.matmul(
                            ot_ps[:, b * BLOCK : (b + 1) * BLOCK].rearrange(
                                "p (q n) -> p q n", q=GPB
                            ),
                            lhsT=basis_t[:, bb * F : (bb + 1) * F],
                            rhs=gt_sbs[b][:, :, bb, :],
                            start=(bb == 0 and b == 0),
                            stop=(bb == NB - 1 and b == BPS - 1),
                            skip_group_check=True,
                        )
                ot_sb = otp.tile([F, BPS * BLOCK], F32)
                nc.scalar.activation(
                    ot_sb[:],
                    ot_ps[:],
                    mybir.ActivationFunctionType.Identity,
                    bias=bias_col[:],
                )
                nc.sync.dma_start(out=outT[sb, :, :], in_=ot_sb[:])

    nc.compile()
    return nc


def _preprocess(basis_coeff, edge_val, edge_src, edge_dst):
    """Pack edges into the static (SB, segment, column) structure.
    Returns per-core (eidx [128, NSB*IDX_COLS] int16,
    meta [128, NSB*META_COLS] bf16, gcnt [128, NSB*NSEG] int32)."""
    src = np.ascontiguousarray(edge_src).ravel()
    dst = np.ascontiguousarray(edge_dst).ravel()
    val = np.ascontiguousarray(edge_val).ravel().astype(np.float32)
    rel = np.repeat(np.arange(NREL, dtype=np.int32), edge_src.shape[1])
    coeff = np.asarray(basis_coeff, dtype=np.float32)  # [NREL, NB]

    core = dst // NS
    per_core = []
    n_grp = NBLK * GPB  # 400 padded group slots (391 real)
    for c in range(NCORES):
        msel = core == c
        s_ = src[msel]
        dl = dst[msel] - c * NS
        v = val[msel]
        r = rel[msel]

        g = dl // GROUP                  # group 0..390
        w = (dl % GROUP).astype(np.float32)
        seg = s_ // SEG                  # 0..3
        lidx = (s_ % SEG + 1).astype(np.int16)  # 1..25000 (0 = zero row)

        bucket = g.astype(np.int64) * NSEG + seg
        order = np.argsort(bucket, kind="stable")
        s_, dl, v, r, g, w, seg, lidx, bucket = (
            a[order] for a in (s_, dl, v, r, g, w, seg, lidx, bucket)
        )
        cnt = np.bincount(bucket, minlength=n_grp * NSEG)
        starts = np.zeros(n_grp * NSEG + 1, dtype=np.int64)
        np.cumsum(cnt, out=starts[1:])
        pos = np.arange(len(s_)) - starts[bucket]

        j = g // GPB                     # block 0..97
        q = g % GPB
        sbi = j // BPS
        b_in = j % BPS                   # block within SB
        bis = b_in * GPB + q             # base column index, 0..15

        in128 = pos < 128
        # overflow edges: packed densely into shared column 16 (blocks 0-1)
        # or 17 (blocks 2-3), ordered by (block, q, pos); the stable sort by
        # bucket gives that order once regrouped by (SB, seg, pair)
        pair = (b_in >= 2).astype(np.int64)
        ov_idx = np.nonzero(~in128)[0]
        ov_key = (sbi[ov_idx] * NSEG + seg[ov_idx]) * 2 + pair[ov_idx]
        ov_order = np.argsort(ov_key, kind="stable")
        ov_sorted = ov_idx[ov_order]
        ov_key_sorted = ov_key[ov_order]
        ov_cnt = np.bincount(ov_key_sorted, minlength=NSB * NSEG * 2)
        assert ov_cnt.max() <= 128, f"overflow column exceeded: {ov_cnt.max()}"
        ov_starts = np.zeros(NSB * NSEG * 2 + 1, dtype=np.int64)
        np.cumsum(ov_cnt, out=ov_starts[1:])
        ovpos = np.zeros(len(s_), dtype=np.int64)
        ovpos[ov_sorted] = np.arange(len(ov_sorted)) - ov_starts[ov_key_sorted]

        col = np.where(in128, bis, 16 + pair)          # physical col 0..17
        part = np.where(in128, pos, ovpos)
        vc = np.where(in128, bis, 16 + 4 * b_in + q)   # virtual col 0..31
        gpos = col * 128 + part

        # ---- index array: valid prefix + trailing -1; per-(SB,seg) counts
        idx_flat = np.zeros((NSB, NSEG, SEG_IDX), dtype=np.int16)
        idx_flat[sbi, seg, gpos] = lidx
        maxg = np.full((NSB, NSEG), -1, dtype=np.int64)
        np.maximum.at(maxg, (sbi, seg), gpos)
        # valid-prefix length, clamped so trimming stays within the two
        # overflow columns (base columns are always fully gathered)
        maxg = np.maximum(maxg, 16 * 128 - 1)
        trail = np.arange(SEG_IDX)[None, None, :] > maxg[:, :, None]
        idx_flat[trail] = -1
        counts = (maxg + 1).astype(np.int32)

        # wrap: position i = s16*16 + p16 -> [16, SEG_IDX//16], tiled to 128
        wrapped = idx_flat.reshape(NSB, NSEG, SEG_IDX // 16, 16).transpose(0, 1, 3, 2)
        wrapped = np.broadcast_to(
            wrapped[:, :, None, :, :], (NSB, NSEG, 8, 16, SEG_IDX // 16)
        ).reshape(NSB, NSEG, 128, SEG_IDX // 16)
        eidx_c = np.ascontiguousarray(
            wrapped.transpose(2, 0, 1, 3).reshape(128, NSB * IDX_COLS)
        )

        # ---- meta per SB: [ldst: SC][w4: SC*NB], sc = seg*CV + vc, bf16
        scix = seg * CV + vc
        mldst = np.zeros((NSB, 128, SC), dtype=np.float32)
        mw4 = np.zeros((NSB, 128, SC, NB), dtype=np.float32)
        mldst[sbi, part, scix] = w
        mw4[sbi, part, scix] = v[:, None] * coeff[r]
        meta_c = np.concatenate(
            [mldst, mw4.reshape(NSB, 128, SC * NB)], axis=2
        )  # [NSB, 128, META_COLS]
        meta_c = np.ascontiguousarray(
            meta_c.transpose(1, 0, 2).reshape(128, NSB * META_COLS)
        ).astype(NPBF16)

        gcnt_c = np.ascontiguousarray(
            np.broadcast_to(
                counts.reshape(1, NSB * NSEG), (128, NSB * NSEG)
            )
        ).astype(np.int32)
        per_core.append((eidx_c, meta_c, gcnt_c))
    return per_core


def _build_table(inp):
    tbl = np.zeros((TBL_ROWS, F), dtype=np.float32)
    for s in range(NSEG):
        tbl[s * (SEG + 1) + 1 : (s + 1) * (SEG + 1)] = inp[s * SEG : (s + 1) * SEG]
    return tbl.astype(NPBF16)


def kernel(inp, basis_weights, basis_coeff, bias, edge_val, edge_src, edge_dst):
    inp = np.ascontiguousarray(np.asarray(inp, dtype=np.float32))
    basis_weights = np.ascontiguousarray(np.asarray(basis_weights, dtype=np.float32))
    basis_coeff = np.asarray(basis_coeff, dtype=np.float32)
    bias = np.ascontiguousarray(np.asarray(bias, dtype=np.float32))

    if "nc" not in _compiled:
        _compiled["nc"] = _build_program()
    nc = _compiled["nc"]

    per_core = _preprocess(basis_coeff, edge_val, edge_src, edge_dst)
    tbl = _build_table(inp)
    iota_np = np.ascontiguousarray(
        np.arange(GROUP, dtype=np.float32)[None, :].repeat(128, 0)
    ).astype(NPBF16)
    basis_bf = basis_weights.astype(NPBF16)

    in_maps = []
    for c in range(NCORES):
        eidx_c, meta_c, gcnt_c = per_core[c]
        in_maps.append(
            {
                "tbl": tbl,
                "basisw": basis_bf,
                "biasw": bias,
                "iota": iota_np,
                "eidx": eidx_c,
                "meta": meta_c,
                "gcnt": gcnt_c,
            }
        )

    res = run_bass_kernel_spmd(nc, in_maps, list(range(NCORES)))
    _compiled["last_results"] = res

    out = np.empty((NN, F), dtype=np.float32)
    for c in range(NCORES):
        oT = res.results[c]["outT"]  # [NSB, F, BPS*BLOCK]
        rows = oT.transpose(0, 2, 1).reshape(NBLK * BLOCK, F)[:NS]
        out[c * NS : (c + 1) * NS] = rows
    return out
